# revision 4
# baseline (speedup 1.0000x reference)
"""Trainium2 Bass kernel for nn_LogicLayer — v2.

Math: out[b,o] = sum_f softmax(weights[o])[f] * op_f(a,b),
      a = x[b, idx0[o]], b = x[b, idx1[o]].
All 16 logic ops are affine in {1, a, b, ab}, so
      out[b,o] = C0[o] + CA[o]*a + CB[o]*b + CAB[o]*a*b.

v2 changes vs v1:
 - softmax -> coefficients computed on HOST; device uploads C0/CA/CB/CAB
   directly (f32, 133KB vs 532KB for raw weights; kills the device
   softmax preamble ~10us of DVE/ACT/sem churn).
 - gather calls are 512 indices (two a-calls + two b-calls per 1024-column
   chunk); with the doubled dynamic-DMA scratch carveout (32KB) this was
   the fastest measured SWDGE configuration.
 - compute per slot (128 neurons x 256 batch): t1 = CAB*a + CB and
   t2 = CA*a + C0 as fused tensor_scalar (DVE) / activation (ACT),
   whole slots assigned to DVE vs ACT by a bitmask knob to balance the
   engines; then chunk-wide DVE tensor_tensor out = t1*b + t2.
"""

import os

import numpy as np

B = 256
IN_DIM = 65536
OUT_DIM = 65536
NFN = 16
NCORES = 8
SHARD = OUT_DIM // NCORES
HALF = IN_DIM // 2
P = 128

# Coefficient table: op_f(a,b) = T[f,0] + T[f,1]*a + T[f,2]*b + T[f,3]*ab
_T = np.array(
    [
        [0, 0, 0, 0],    # false
        [0, 0, 0, 1],    # a AND b
        [0, 1, 0, -1],   # a AND NOT b
        [0, 1, 0, 0],    # a
        [0, 0, 1, -1],   # NOT a AND b
        [0, 0, 1, 0],    # b
        [0, 1, 1, -2],   # XOR
        [0, 1, 1, -1],   # OR
        [1, -1, -1, 1],  # NOR
        [1, -1, -1, 2],  # XNOR
        [1, 0, -1, 0],   # NOT b
        [1, 0, -1, 1],   # a OR NOT b
        [1, -1, 0, 0],   # NOT a
        [1, -1, 0, 1],   # NOT a OR b
        [1, 0, 0, -1],   # NAND
        [1, 0, 0, 0],    # true
    ],
    dtype=np.float32,
)

_BUILD_CACHE = {}
LAST_RESULTS = None  # BassKernelResults of the most recent run (for profiling)


def _wrap_idx(idx16):
    """[n] int16 -> [128, n//16] wrapped: position i at (i%16, i//16),
    replicated across the 8 groups of 16 partitions (one per Q7 core)."""
    w = idx16.reshape(-1, 16).T  # [16, n/16]
    return np.ascontiguousarray(np.tile(w, (8, 1)))


def _build_kernel(caps):
    """Build + compile the SPMD program for bucket capacities `caps` (4-tuple,
    each a multiple of 128). Returns (nc, npad)."""
    key = tuple(caps)
    if key in _BUILD_CACHE:
        return _BUILD_CACHE[key]

    import concourse.bacc as bacc
    import concourse.mybir as mybir
    import concourse.tile as tile
    from concourse import library_config

    npad = int(sum(caps))
    nslot = npad // P
    offs = np.concatenate([[0], np.cumsum(caps)]).astype(int)

    nc = bacc.Bacc(
        "TRN2",
        target_bir_lowering=False,
        debug=False,
        dynamic_dma_scratch_size=int(os.environ.get("K_DMA_SCRATCH", "32768")),
        num_swdge_queues=4,
    )
    f32 = mybir.dt.float32
    f16 = mybir.dt.float16
    i16 = mybir.dt.int16

    xA_d = nc.dram_tensor("xA", [HALF, B], f16, kind="ExternalInput")
    xB_d = nc.dram_tensor("xB", [HALF, B], f16, kind="ExternalInput")
    # combined index array: buckets hold [ia_k ; ib_k] back to back.
    ix_d = nc.dram_tensor("ix", [P, 2 * npad // 16], i16, kind="ExternalInput")
    # coefficients, host-precomputed from softmax(weights):
    # [P, 4*nslot] f32, j-major: [c0 | ca | cb | cab], each wrapped [P, nslot]
    c_d = nc.dram_tensor("coef", [P, 4 * nslot], f32, kind="ExternalInput")
    out_d = nc.dram_tensor("out", [P, nslot * B], f16, kind="ExternalOutput")

    Ident = mybir.ActivationFunctionType.Identity
    Mult = mybir.AluOpType.mult
    Add = mybir.AluOpType.add

    MAX_CALL = int(os.environ.get("K_MAX_CALL", "512"))

    def qsel(r):
        return r % 4
    SP = os.environ.get("K_SINGLE_PACKET", "1") == "1"
    # bitmask over slot%8: bit set -> both TS of that slot run on DVE,
    # else on ACT. Default 10101010b -> 4/8 slots on DVE.
    DPAT = int(os.environ.get("K_DPAT", "0b01010101"), 0)

    from contextlib import ExitStack

    with tile.TileContext(nc) as tc, ExitStack() as ctx:
        nc.gpsimd.load_library(library_config.mlp)
        consts = ctx.enter_context(tc.tile_pool(name="consts", bufs=1))
        work = ctx.enter_context(
            tc.tile_pool(name="work", bufs=int(os.environ.get("K_BUFS", "8")))
        )

        # --- load index lists (stay resident) ---
        ix_t = consts.tile([P, 2 * npad // 16], i16)
        nc.sync.dma_start(out=ix_t[:], in_=ix_d[:])

        # --- warmup: one tiny gather absorbs the Q7 library IRAM load while
        # the index DMAs land ---
        warm_i = consts.tile([P, 8], i16)
        nc.vector.memset(warm_i[:], 0)
        warm_o = consts.tile([P, 1, B], f16)
        nc.gpsimd.dma_gather(
            out_ap=warm_o[:],
            in_ap=xA_d[:],
            idxs_ap=warm_i[:],
            num_idxs=P,
            num_idxs_reg=P,
            elem_size=B,
            single_packet=True,
            queue_num=0,
        )

        # --- coefficients (host-precomputed) ---
        c_t = consts.tile([P, 4 * nslot], f32)
        nc.sync.dma_start(out=c_t[:], in_=c_d[:])

        def coef(j, g):
            return c_t[:, j * nslot + g : j * nslot + g + 1]

        # --- main loop over bucket-aligned chunks of columns ---
        chunk_cap = int(os.environ.get("K_CHUNK_POS", "1024"))
        chunks = []  # (bucket, p0, p1)
        tail_sz = int(os.environ.get("K_TAIL", "0"))
        for k in range(4):
            cap = int(caps[k])
            if cap == 0:
                continue
            tail = tail_sz if k == 3 and cap >= tail_sz + P else 0
            cap -= tail
            nch = max(1, -(-cap // chunk_cap))
            lo = int(offs[k])
            for i in range(nch):
                sz = P * (
                    (cap * (i + 1)) // (nch * P) - (cap * i) // (nch * P)
                )
                chunks.append((k, lo, lo + sz))
                lo += sz
            if tail:
                chunks.append((k, lo, lo + tail))
                lo += tail
            assert lo == offs[k + 1]
        qrot = 0
        TAILSPLIT = os.environ.get("K_TAILSPLIT", "1") == "1"
        for ci, (k, p0g, p1g) in enumerate(chunks):
            is_last = ci == len(chunks) - 1
            cbase, cs = p0g // P, (p1g - p0g) // P
            src_a = xA_d if k < 2 else xB_d
            src_b = xA_d if k % 2 == 0 else xB_d
            t1_t = work.tile([P, cs, B], f16)
            t2_t = work.tile([P, cs, B], f16)
            a_t = work.tile([P, cs, B], f16)
            b_t = work.tile([P, cs, B], f16)
            ia0 = 2 * offs[k] + (p0g - offs[k])
            ib0 = 2 * offs[k] + int(caps[k]) + (p0g - offs[k])
            n_tot = p1g - p0g
            # a-call(s) first: the per-slot t1/t2 compute depends only on a,
            # so it can start while the b-call is still draining
            for (dst, src, i0) in ((a_t, src_a, ia0), (b_t, src_b, ib0)):
                lo = 0
                while lo < n_tot:
                    n = min(MAX_CALL, n_tot - lo)
                    sl, sh = lo // P, (lo + n) // P
                    nc.gpsimd.dma_gather(
                        out_ap=dst[:, sl:sh, :],
                        in_ap=src[:],
                        idxs_ap=ix_t[
                            :, (i0 + lo) // 16 : (i0 + lo + n) // 16
                        ],
                        num_idxs=n,
                        num_idxs_reg=n,
                        elem_size=B,
                        single_packet=SP,
                        queue_num=qsel(qrot),
                    )
                    qrot += 1
                    lo += n

            for s in range(cs):
                g = cbase + s
                a_s = a_t[:, s, :]
                if (DPAT >> (s % 8)) & 1:
                    # both fused scale+bias ops on DVE
                    nc.vector.tensor_scalar(
                        t1_t[:, s, :], a_s, coef(3, g), coef(2, g), Mult, Add
                    )
                    nc.vector.tensor_scalar(
                        t2_t[:, s, :], a_s, coef(1, g), coef(0, g), Mult, Add
                    )
                else:
                    # both on ACT
                    nc.scalar.activation(
                        t1_t[:, s, :], a_s, Ident,
                        bias=coef(2, g), scale=coef(3, g),
                    )
                    nc.scalar.activation(
                        t2_t[:, s, :], a_s, Ident,
                        bias=coef(0, g), scale=coef(1, g),
                    )
            # out = t1*b + t2   (DVE, chunk-wide, in place into t1). The last
            # chunk combines+writes in two halves so the final store overlaps
            # the final compute (shorter kernel tail).
            halves = (
                [(0, cs // 2), (cs // 2, cs)]
                if (TAILSPLIT and is_last and cs >= 2)
                else [(0, cs)]
            )
            for (h0, h1) in halves:
                nc.vector.tensor_mul(
                    t1_t[:, h0:h1, :], t1_t[:, h0:h1, :], b_t[:, h0:h1, :]
                )
                nc.vector.tensor_add(
                    t1_t[:, h0:h1, :], t1_t[:, h0:h1, :], t2_t[:, h0:h1, :]
                )
                nc.sync.dma_start(
                    out=out_d[:, (cbase + h0) * B : (cbase + h1) * B],
                    in_=t1_t[:, h0:h1, :].rearrange("p s e -> p (s e)"),
                )

    nc.compile()
    _BUILD_CACHE[key] = (nc, npad)
    return nc, npad


def kernel(x, weights, indices):
    from concourse.bass_utils import run_bass_kernel_spmd

    x = np.asarray(x, dtype=np.float32)
    weights = np.asarray(weights, dtype=np.float32)
    indices = np.asarray(indices, dtype=np.int64)

    x_T = np.ascontiguousarray(x.T.astype(np.float16))  # [IN_DIM, B] fp16
    xA = x_T[:HALF]
    xB = x_T[HALF:]

    # host softmax -> affine coefficients [OUT_DIM, 4] (j: c0, ca, cb, cab)
    e = np.exp(weights - weights.max(-1, keepdims=True))
    probs = e / e.sum(-1, keepdims=True)
    coefs = probs @ _T  # [OUT_DIM, 4]

    # --- global bucketing: columns are dealt to cores per bucket so every
    # core gets near-equal bucket counts (the host unpermutes outputs, so any
    # column->core assignment is valid). ---
    bid_all = (indices[0] >= HALF).astype(np.int64) * 2 + (
        indices[1] >= HALF
    ).astype(np.int64)
    percore_cols = [[None] * 4 for _ in range(NCORES)]
    counts_all = np.zeros((NCORES, 4), dtype=np.int64)
    for k in range(4):
        cols_k = np.nonzero(bid_all == k)[0]
        for c, part in enumerate(np.array_split(cols_k, NCORES)):
            percore_cols[c][k] = part
            counts_all[c, k] = len(part)

    caps = tuple(
        int(-(-int(counts_all[:, k].max()) // P) * P) for k in range(4)
    )
    nc, npad = _build_kernel(caps)
    nslot = npad // P
    offs = np.concatenate([[0], np.cumsum(caps)]).astype(int)

    in_maps = []
    pos_maps = []  # per core: global column index per position (-1 = pad)
    for c in range(NCORES):
        ia = np.zeros(npad, dtype=np.int16)
        ib = np.zeros(npad, dtype=np.int16)
        pos = np.full(npad, -1, dtype=np.int64)
        c_pad = np.zeros((npad, 4), dtype=np.float32)
        for k in range(4):
            cols = percore_cols[c][k]
            o, n = int(offs[k]), len(cols)
            ia[o : o + n] = (
                indices[0, cols] - (HALF if k >= 2 else 0)
            ).astype(np.int16)
            ib[o : o + n] = (
                indices[1, cols] - (HALF if k % 2 else 0)
            ).astype(np.int16)
            pos[o : o + n] = cols
            c_pad[o : o + n] = coefs[cols]
        # combined index array: buckets store [ia_k ; ib_k].
        ix = np.zeros(2 * npad, dtype=np.int16)
        for k in range(4):
            o, cap = int(offs[k]), int(caps[k])
            ix[2 * o : 2 * o + cap] = ia[o : o + cap]
            ix[2 * o + cap : 2 * o + 2 * cap] = ib[o : o + cap]
        # wrap coefficients to [P, 4*nslot] j-major: position i = s*128 + p
        # -> partition p, col j*nslot + s
        c_wrapped = np.ascontiguousarray(
            c_pad.reshape(nslot, P, 4).transpose(1, 2, 0)
        ).reshape(P, 4 * nslot)
        in_maps.append(
            {
                "xA": xA,
                "xB": xB,
                "ix": _wrap_idx(ix),
                "coef": c_wrapped,
            }
        )
        pos_maps.append(pos)

    res = run_bass_kernel_spmd(nc, in_maps, core_ids=list(range(NCORES)))
    global LAST_RESULTS
    LAST_RESULTS = res

    out = np.empty((B, OUT_DIM), dtype=np.float32)
    for c in range(NCORES):
        o = res.results[c]["out"].reshape(P, nslot, B).astype(np.float32)
        rows = np.ascontiguousarray(o.transpose(1, 0, 2)).reshape(npad, B)
        pos = pos_maps[c]
        valid = pos >= 0
        out[:, pos[valid]] = rows[valid].T
    return out


# revision 15
# speedup vs baseline: 1.0221x; 1.0221x over previous
"""Trainium2 Bass kernel for nn_LogicLayer — v2.

Math: out[b,o] = sum_f softmax(weights[o])[f] * op_f(a,b),
      a = x[b, idx0[o]], b = x[b, idx1[o]].
All 16 logic ops are affine in {1, a, b, ab}, so
      out[b,o] = C0[o] + CA[o]*a + CB[o]*b + CAB[o]*a*b.

v2 changes vs v1:
 - softmax -> coefficients computed on HOST; device uploads C0/CA/CB/CAB
   directly (f32, 133KB vs 532KB for raw weights; kills the device
   softmax preamble ~10us of DVE/ACT/sem churn).
 - gather calls are 512 indices (two a-calls + two b-calls per 1024-column
   chunk); with the doubled dynamic-DMA scratch carveout (32KB) this was
   the fastest measured SWDGE configuration.
 - compute per slot (128 neurons x 256 batch): t1 = CAB*a + CB and
   t2 = CA*a + C0 as fused tensor_scalar (DVE) / activation (ACT),
   whole slots assigned to DVE vs ACT by a bitmask knob to balance the
   engines; then chunk-wide DVE tensor_tensor out = t1*b + t2.
"""

import os

import numpy as np

B = 256
IN_DIM = 65536
OUT_DIM = 65536
NFN = 16
NCORES = 8
SHARD = OUT_DIM // NCORES
HALF = IN_DIM // 2
P = 128

# Coefficient table: op_f(a,b) = T[f,0] + T[f,1]*a + T[f,2]*b + T[f,3]*ab
_T = np.array(
    [
        [0, 0, 0, 0],    # false
        [0, 0, 0, 1],    # a AND b
        [0, 1, 0, -1],   # a AND NOT b
        [0, 1, 0, 0],    # a
        [0, 0, 1, -1],   # NOT a AND b
        [0, 0, 1, 0],    # b
        [0, 1, 1, -2],   # XOR
        [0, 1, 1, -1],   # OR
        [1, -1, -1, 1],  # NOR
        [1, -1, -1, 2],  # XNOR
        [1, 0, -1, 0],   # NOT b
        [1, 0, -1, 1],   # a OR NOT b
        [1, -1, 0, 0],   # NOT a
        [1, -1, 0, 1],   # NOT a OR b
        [1, 0, 0, -1],   # NAND
        [1, 0, 0, 0],    # true
    ],
    dtype=np.float32,
)

_BUILD_CACHE = {}
LAST_RESULTS = None  # BassKernelResults of the most recent run (for profiling)


def _wrap_idx(idx16):
    """[n] int16 -> [128, n//16] wrapped: position i at (i%16, i//16),
    replicated across the 8 groups of 16 partitions (one per Q7 core)."""
    w = idx16.reshape(-1, 16).T  # [16, n/16]
    return np.ascontiguousarray(np.tile(w, (8, 1)))


def _build_kernel(caps):
    """Build + compile the SPMD program for bucket capacities `caps` (4-tuple,
    each a multiple of 128). Returns (nc, npad)."""
    key = tuple(caps)
    if key in _BUILD_CACHE:
        return _BUILD_CACHE[key]

    import concourse.bacc as bacc
    import concourse.mybir as mybir
    import concourse.tile as tile
    from concourse import library_config

    npad = int(sum(caps))
    nslot = npad // P
    offs = np.concatenate([[0], np.cumsum(caps)]).astype(int)

    nc = bacc.Bacc(
        "TRN2",
        target_bir_lowering=False,
        debug=False,
        dynamic_dma_scratch_size=int(os.environ.get("K_DMA_SCRATCH", "32768")),
        num_swdge_queues=4,
    )
    f32 = mybir.dt.float32
    f16 = mybir.dt.float16
    i16 = mybir.dt.int16

    xA_d = nc.dram_tensor("xA", [HALF, B], f16, kind="ExternalInput")
    xB_d = nc.dram_tensor("xB", [HALF, B], f16, kind="ExternalInput")
    # combined index array: buckets hold [ia_k ; ib_k] back to back.
    ix_d = nc.dram_tensor("ix", [P, 2 * npad // 16], i16, kind="ExternalInput")
    # coefficients, host-precomputed from softmax(weights):
    # [P, 4*nslot] f32, j-major: [c0 | ca | cb | cab], each wrapped [P, nslot]
    c_d = nc.dram_tensor("coef", [P, 4 * nslot], f32, kind="ExternalInput")
    out_d = nc.dram_tensor("out", [P, nslot * B], f16, kind="ExternalOutput")

    Ident = mybir.ActivationFunctionType.Identity
    Mult = mybir.AluOpType.mult
    Add = mybir.AluOpType.add

    MAX_CALL = int(os.environ.get("K_MAX_CALL", "512"))
    # prepare_only + trigger_dma: the prep instruction retires at
    # descriptor-generation time instead of DMA completion, so the Pool
    # engine's 4-deep exec queue no longer caps in-flight gather DMAs.
    PREP = os.environ.get("K_PREP", "0") == "1"

    def qsel(r):
        return r % 4
    SP = os.environ.get("K_SINGLE_PACKET", "1") == "1"
    # bitmask over slot%8: bit set -> both TS of that slot run on DVE,
    # else on ACT. Default 10101010b -> 4/8 slots on DVE.
    DPAT = int(os.environ.get("K_DPAT", "0b01010101"), 0)

    from contextlib import ExitStack

    with tile.TileContext(nc) as tc, ExitStack() as ctx:
        nc.gpsimd.load_library(library_config.mlp)
        consts = ctx.enter_context(tc.tile_pool(name="consts", bufs=1))
        work = ctx.enter_context(
            tc.tile_pool(name="work", bufs=int(os.environ.get("K_BUFS", "8")))
        )

        # --- load index lists (stay resident) ---
        ix_t = consts.tile([P, 2 * npad // 16], i16)
        nc.sync.dma_start(out=ix_t[:], in_=ix_d[:])

        # --- warmup: one tiny gather absorbs the Q7 library IRAM load while
        # the index DMAs land ---
        warm_i = consts.tile([P, 8], i16)
        nc.vector.memset(warm_i[:], 0)
        warm_o = consts.tile([P, 1, B], f16)
        nc.gpsimd.dma_gather(
            out_ap=warm_o[:],
            in_ap=xA_d[:],
            idxs_ap=warm_i[:],
            num_idxs=P,
            num_idxs_reg=P,
            elem_size=B,
            single_packet=True,
            queue_num=0,
        )

        # --- coefficients (host-precomputed) ---
        c_t = consts.tile([P, 4 * nslot], f32)
        nc.sync.dma_start(out=c_t[:], in_=c_d[:])

        def coef(j, g):
            return c_t[:, j * nslot + g : j * nslot + g + 1]

        # --- main loop over bucket-aligned chunks of columns ---
        chunk_cap = int(os.environ.get("K_CHUNK_POS", "1024"))
        chunks = []  # (bucket, p0, p1)
        tail_sz = int(os.environ.get("K_TAIL", "0"))
        for k in range(4):
            cap = int(caps[k])
            if cap == 0:
                continue
            tail = tail_sz if k == 3 and cap >= tail_sz + P else 0
            cap -= tail
            nch = max(1, -(-cap // chunk_cap))
            lo = int(offs[k])
            for i in range(nch):
                sz = P * (
                    (cap * (i + 1)) // (nch * P) - (cap * i) // (nch * P)
                )
                chunks.append((k, lo, lo + sz))
                lo += sz
            if tail:
                chunks.append((k, lo, lo + tail))
                lo += tail
            assert lo == offs[k + 1]
        qrot = 0
        TAILSPLIT = os.environ.get("K_TAILSPLIT", "1") == "1"
        # prepare_only wiring: the prep instruction retires at descriptor-gen
        # time, so Tile's auto-generated consumer waits fire too early. We
        # attach our own per-queue DMA-completion sem (each call +16; calls
        # on one queue complete in FIFO order) and add explicit engine-level
        # wait_ge on the consumers before they read gathered tiles.
        gsems = None
        qcnt = [0, 0, 0, 0]  # completed-call targets per queue
        if PREP:
            gsems = [nc.alloc_semaphore(f"gq{q}") for q in range(4)]
            for s in gsems:
                nc.gpsimd.sem_clear(s)
        for ci, (k, p0g, p1g) in enumerate(chunks):
            is_last = ci == len(chunks) - 1
            cbase, cs = p0g // P, (p1g - p0g) // P
            src_a = xA_d if k < 2 else xB_d
            src_b = xA_d if k % 2 == 0 else xB_d
            t1_t = work.tile([P, cs, B], f16)
            t2_t = work.tile([P, cs, B], f16)
            a_t = work.tile([P, cs, B], f16)
            b_t = work.tile([P, cs, B], f16)
            ia0 = 2 * offs[k] + (p0g - offs[k])
            ib0 = 2 * offs[k] + int(caps[k]) + (p0g - offs[k])
            n_tot = p1g - p0g
            # a-call(s) first: the per-slot t1/t2 compute depends only on a,
            # so it can start while the b-call is still draining
            wait_a, wait_b = [], []  # (queue, sem target) per stream
            for (dst, src, i0, wl) in (
                (a_t, src_a, ia0, wait_a),
                (b_t, src_b, ib0, wait_b),
            ):
                lo = 0
                while lo < n_tot:
                    n = min(MAX_CALL, n_tot - lo)
                    sl, sh = lo // P, (lo + n) // P
                    q = qsel(qrot)
                    kw = dict(
                        out_ap=dst[:, sl:sh, :],
                        in_ap=src[:],
                        idxs_ap=ix_t[
                            :, (i0 + lo) // 16 : (i0 + lo + n) // 16
                        ],
                        num_idxs=n,
                        num_idxs_reg=n,
                        elem_size=B,
                        single_packet=SP,
                        queue_num=q,
                    )
                    if PREP:
                        nc.gpsimd.dma_gather(
                            prepare_only=True, sem=gsems[q], **kw
                        )
                        nc.gpsimd.trigger_dma(count=None, queue_num=q)
                        qcnt[q] += 1
                        wl.append((q, 16 * qcnt[q]))
                    else:
                        nc.gpsimd.dma_gather(**kw)
                    qrot += 1
                    lo += n
            if PREP:
                # gate consumers on actual DMA completion of their inputs
                for (q, tgt) in wait_a:
                    nc.vector.wait_ge(gsems[q], tgt)
                    nc.scalar.wait_ge(gsems[q], tgt)

            for s in range(cs):
                g = cbase + s
                a_s = a_t[:, s, :]
                if (DPAT >> (s % 8)) & 1:
                    # both fused scale+bias ops on DVE
                    nc.vector.tensor_scalar(
                        t1_t[:, s, :], a_s, coef(3, g), coef(2, g), Mult, Add
                    )
                    nc.vector.tensor_scalar(
                        t2_t[:, s, :], a_s, coef(1, g), coef(0, g), Mult, Add
                    )
                else:
                    # both on ACT
                    nc.scalar.activation(
                        t1_t[:, s, :], a_s, Ident,
                        bias=coef(2, g), scale=coef(3, g),
                    )
                    nc.scalar.activation(
                        t2_t[:, s, :], a_s, Ident,
                        bias=coef(0, g), scale=coef(1, g),
                    )
            # out = t1*b + t2   (DVE, chunk-wide, in place into t1). The last
            # chunk combines+writes in two halves so the final store overlaps
            # the final compute (shorter kernel tail).
            if PREP:
                for (q, tgt) in wait_b:
                    nc.vector.wait_ge(gsems[q], tgt)
            halves = (
                [(0, cs // 2), (cs // 2, cs)]
                if (TAILSPLIT and is_last and cs >= 2)
                else [(0, cs)]
            )
            for (h0, h1) in halves:
                nc.vector.tensor_mul(
                    t1_t[:, h0:h1, :], t1_t[:, h0:h1, :], b_t[:, h0:h1, :]
                )
                nc.vector.tensor_add(
                    t1_t[:, h0:h1, :], t1_t[:, h0:h1, :], t2_t[:, h0:h1, :]
                )
                nc.sync.dma_start(
                    out=out_d[:, (cbase + h0) * B : (cbase + h1) * B],
                    in_=t1_t[:, h0:h1, :].rearrange("p s e -> p (s e)"),
                )

    nc.compile()
    _BUILD_CACHE[key] = (nc, npad)
    return nc, npad


def kernel(x, weights, indices):
    from concourse.bass_utils import run_bass_kernel_spmd

    x = np.asarray(x, dtype=np.float32)
    weights = np.asarray(weights, dtype=np.float32)
    indices = np.asarray(indices, dtype=np.int64)

    x_T = np.ascontiguousarray(x.T.astype(np.float16))  # [IN_DIM, B] fp16
    xA = x_T[:HALF]
    xB = x_T[HALF:]

    # host softmax -> affine coefficients [OUT_DIM, 4] (j: c0, ca, cb, cab)
    e = np.exp(weights - weights.max(-1, keepdims=True))
    probs = e / e.sum(-1, keepdims=True)
    coefs = probs @ _T  # [OUT_DIM, 4]

    # --- global bucketing: columns are dealt to cores per bucket so every
    # core gets near-equal bucket counts (the host unpermutes outputs, so any
    # column->core assignment is valid). ---
    bid_all = (indices[0] >= HALF).astype(np.int64) * 2 + (
        indices[1] >= HALF
    ).astype(np.int64)
    percore_cols = [[None] * 4 for _ in range(NCORES)]
    counts_all = np.zeros((NCORES, 4), dtype=np.int64)
    for k in range(4):
        cols_k = np.nonzero(bid_all == k)[0]
        for c, part in enumerate(np.array_split(cols_k, NCORES)):
            percore_cols[c][k] = part
            counts_all[c, k] = len(part)

    caps = tuple(
        int(-(-int(counts_all[:, k].max()) // P) * P) for k in range(4)
    )
    nc, npad = _build_kernel(caps)
    nslot = npad // P
    offs = np.concatenate([[0], np.cumsum(caps)]).astype(int)

    in_maps = []
    pos_maps = []  # per core: global column index per position (-1 = pad)
    for c in range(NCORES):
        ia = np.zeros(npad, dtype=np.int16)
        ib = np.zeros(npad, dtype=np.int16)
        pos = np.full(npad, -1, dtype=np.int64)
        c_pad = np.zeros((npad, 4), dtype=np.float32)
        for k in range(4):
            cols = percore_cols[c][k]
            o, n = int(offs[k]), len(cols)
            ia[o : o + n] = (
                indices[0, cols] - (HALF if k >= 2 else 0)
            ).astype(np.int16)
            ib[o : o + n] = (
                indices[1, cols] - (HALF if k % 2 else 0)
            ).astype(np.int16)
            pos[o : o + n] = cols
            c_pad[o : o + n] = coefs[cols]
        # combined index array: buckets store [ia_k ; ib_k].
        ix = np.zeros(2 * npad, dtype=np.int16)
        for k in range(4):
            o, cap = int(offs[k]), int(caps[k])
            ix[2 * o : 2 * o + cap] = ia[o : o + cap]
            ix[2 * o + cap : 2 * o + 2 * cap] = ib[o : o + cap]
        # wrap coefficients to [P, 4*nslot] j-major: position i = s*128 + p
        # -> partition p, col j*nslot + s
        c_wrapped = np.ascontiguousarray(
            c_pad.reshape(nslot, P, 4).transpose(1, 2, 0)
        ).reshape(P, 4 * nslot)
        in_maps.append(
            {
                "xA": xA,
                "xB": xB,
                "ix": _wrap_idx(ix),
                "coef": c_wrapped,
            }
        )
        pos_maps.append(pos)

    res = run_bass_kernel_spmd(nc, in_maps, core_ids=list(range(NCORES)))
    global LAST_RESULTS
    LAST_RESULTS = res

    out = np.empty((B, OUT_DIM), dtype=np.float32)
    for c in range(NCORES):
        o = res.results[c]["out"].reshape(P, nslot, B).astype(np.float32)
        rows = np.ascontiguousarray(o.transpose(1, 0, 2)).reshape(npad, B)
        pos = pos_maps[c]
        valid = pos >= 0
        out[:, pos[valid]] = rows[valid].T
    return out
